# revision 1
# baseline (speedup 1.0000x reference)
"""Trainium2 Bass kernel for nn_AttentionLayer_48722109006175.

Math: out[b,i,j,h] = softmax_h( leaky_relu( attn_src[b,i,h] + attn_dst[b,j,h]
                                            + edge_dense[b,i,j,h], 0.2 ) )

The edge term is linear in src, so the per-edge scatter collapses:
  edge_dense[b,i,j,:] = cnt[i,j] * (g[b,i,:] - g[b,j,:])
where g = src @ (W_edge @ a_edge) and cnt[i,j] counts (i,j) edges (shared by
all batches).  Likewise attn_src = src @ (W_lin @ a_src), attn_dst =
src @ (W_lin @ a_dst).  So with A = [As|Ad|Ag] [128,12],
sdg = src_b @ A gives all per-node terms.

Per core (one batch b), per 128-row i-tile x 512-col j-block, heads paired:
  PE:  psa = Q = g_i - g_j        (K=2 rank-2 matmul, fp32r)
  DVE: psa = cnt .* psa           (in-place on PSUM)
  PE:  psa += P = s_i + d_j       (K=2 matmul accumulate, fp32)
  ACT: l = prelu(psa, 0.2); m = exp(l)
  POOL: s = sum_h m ; DVE: r ~= 1/s (custom-DVE approx, ~51 ULP)
  DVE/POOL: out[:, j*4+h] = m_h * r  (strided interleave)
Sharding: data-parallel over batch, one batch per NeuronCore.
DMAs are batched (one cnt load / one out store per i-tile) because each
dma_start costs ~0.65us of sequencer issue time.
"""

import numpy as np

B, N, F_IN, H = 8, 1024, 128, 4
JB = 512          # j-block (one PSUM bank at fp32)
NT = N // 128     # 8 i-tiles
NEG_SLOPE = 0.2


def _build_nc(mn_bufs=3, m_bufs=6, ps_bufs=4, o_bufs=3, cnt_bufs=3):
    import concourse.bass as bass
    import concourse.bacc as bacc
    import concourse.mybir as mybir
    import concourse.tile as tile
    from concourse.masks import make_identity

    f32 = mybir.dt.float32
    f32r = mybir.dt.float32r
    AF = mybir.ActivationFunctionType
    OP = mybir.AluOpType

    nc = bacc.Bacc()
    # Reset DMA queues + clear bass-managed semaphores at kernel entry.
    # (Bass only emits this when target_bir_lowering=True; without it, stale
    # semaphore/DMA state from previously-executed NEFFs on the same core
    # races the first tile loads.)
    from concourse.bass import compact_to_ranges
    for sem_range in compact_to_ranges(
        [s for s in nc._kernel_sem_range if s not in nc.barrier_sems]
    ):
        nc.gpsimd.dma_reset(sem_range)
        nc.gpsimd.sem_clear(sem_range)
    nc._nrt_pseudo_barrier()

    src_d = nc.dram_tensor("src", [N, F_IN], f32, kind="ExternalInput")
    cnt_d = nc.dram_tensor("cnt", [N, N], f32, kind="ExternalInput")
    a_d = nc.dram_tensor("A", [F_IN, 12], f32, kind="ExternalInput")
    out_d = nc.dram_tensor("out", [N, N * H], f32, kind="ExternalOutput")

    with tile.TileContext(nc) as tc:
        with tc.tile_pool(name="stage", bufs=1) as stage:
            # Packed staging for the K=2 rank-2 matmuls, head chunks of N:
            #   lhsAll row0 = [s_0..s_3 | g_0..g_3],  row1 = ones
            #   rhsAll row0 = ones,  row1 = [d_0..d_3 | -g_0..-g_3]
            # lhsP_h = lhsAll[:, h*N:], lhsQ_h = lhsAll[:, (4+h)*N:], etc.
            lhsAll = stage.tile([2, 2 * H * N], f32)
            rhsAll = stage.tile([2, 2 * H * N], f32)

            # ---- prologue: srcT via PE transpose, sdgT = (src @ A)^T ----
            with tc.tile_pool(name="pro", bufs=1) as pro, \
                 tc.tile_pool(name="pps", bufs=2, space="PSUM") as pps:
                ones_t = pro.tile([1, 2 * N], f32)
                nc.vector.memset(ones_t[:, 0:N], 1.0)
                nc.vector.memset(ones_t[:, N:], -1.0)
                oap1 = ones_t[0:1, 0:N]
                ones_b = bass.AP(tensor=oap1.tensor, offset=oap1.offset,
                                 ap=[oap1.ap[0], [0, 2 * H], oap1.ap[1]])
                oapm = ones_t[0:1, N:]
                mones_b = bass.AP(tensor=oapm.tensor, offset=oapm.offset,
                                  ap=[oapm.ap[0], [0, H], oapm.ap[1]])
                # lhsAll row1: +1 for P chunks (0..3), -1 for Q chunks (4..7)
                nc.sync.dma_start(lhsAll[1:2, 0:H * N],
                                  bass.AP(tensor=oap1.tensor, offset=oap1.offset,
                                          ap=[oap1.ap[0], [0, H], oap1.ap[1]]))
                nc.sync.dma_start(lhsAll[1:2, H * N:], mones_b)
                nc.sync.dma_start(rhsAll[0:1, :], ones_b)
                ident = pro.tile([128, 128], f32)
                make_identity(nc, ident)
                a_sb = pro.tile([F_IN, 12], f32)
                nc.gpsimd.dma_start(a_sb, a_d[:, :])
                chunks = pro.tile([128, N], f32)
                nc.sync.dma_start(
                    chunks.rearrange("p (c f) -> p c f", c=NT),
                    src_d[:, :].rearrange("(c p) f -> p c f", p=128))
                srcT = pro.tile([128, N], f32)
                sdgT = pro.tile([12, N], f32)
                cview = chunks.rearrange("p (c f) -> p c f", c=NT)
                for c in range(NT):
                    pt = pps.tile([128, 128], f32, tag="pt")
                    nc.tensor.transpose(pt, cview[:, c, :], ident)
                    nc.scalar.copy(srcT[:, c * 128:(c + 1) * 128], pt)
                for half in range(2):
                    ps = pps.tile([12, 512], f32, tag="sdg")
                    nc.tensor.matmul(ps, a_sb,
                                     srcT[:, half * 512:(half + 1) * 512],
                                     start=True, stop=True)
                    nc.scalar.copy(sdgT[:, half * 512:(half + 1) * 512], ps)
                # 4 batched row DMAs fill all staging data rows
                nc.sync.dma_start(lhsAll[0:1, 0:H * N], sdgT[0:4, :])
                nc.sync.dma_start(lhsAll[0:1, H * N:], sdgT[8:12, :])
                nc.sync.dma_start(rhsAll[1:2, 0:H * N], sdgT[4:8, :])
                nc.sync.dma_start(rhsAll[1:2, H * N:], sdgT[8:12, :])

            def lhsQ(h):
                return lhsAll[:, (H + h) * N:(H + h + 1) * N]

            def lhsP(h):
                return lhsAll[:, h * N:(h + 1) * N]

            def rhsQ(h):
                return rhsAll[:, (H + h) * N:(H + h + 1) * N]

            def rhsP(h):
                return rhsAll[:, h * N:(h + 1) * N]

            # ---- main loop ----
            with tc.tile_pool(name="mn", bufs=mn_bufs) as mn, \
                 tc.tile_pool(name="mtiles", bufs=m_bufs) as mpool, \
                 tc.tile_pool(name="ob", bufs=o_bufs) as obp, \
                 tc.tile_pool(name="cntp", bufs=cnt_bufs) as cntp, \
                 tc.tile_pool(name="ps", bufs=ps_bufs, space="PSUM") as psp:
                # recompute i-tile 0 last: its startup-issued version can race
                # semaphore warm-up; the final DRAM write wins.
                order = list(range(NT)) + [0]
                cnt_pref = {}

                def load_cnt(i):
                    t = cntp.tile([128, N], f32, tag="cnt", name=f"cnt{i}")
                    nc.sync.dma_start(t, cnt_d[order[i] * 128:order[i] * 128 + 128, :])
                    return t

                cnt_pref[0] = load_cnt(0)
                for idx, it in enumerate(order):
                    i0 = it * 128
                    cnt_t = cnt_pref.pop(idx)
                    if idx + 1 < len(order):
                        cnt_pref[idx + 1] = load_cnt(idx + 1)
                    o_t = obp.tile([128, N * H], f32, tag="o")
                    o3 = o_t.rearrange("p (j h) -> p j h", h=H)
                    for jb in range(N // JB):
                        j0 = jb * JB
                        mpair = []
                        for hp in range(H // 2):
                            psa = psp.tile([128, 2 * JB], f32, tag="psa")
                            for k in range(2):
                                h = hp * 2 + k
                                nc.tensor.matmul(
                                    psa[:, k * JB:(k + 1) * JB],
                                    lhsQ(h)[:, i0:i0 + 128].bitcast(f32r),
                                    rhsQ(h)[:, j0:j0 + JB].bitcast(f32r),
                                    start=True, stop=True)
                            cs = cnt_t[:, j0:j0 + JB]
                            cnt_b = bass.AP(
                                tensor=cs.tensor, offset=cs.offset,
                                ap=[cs.ap[0], [0, 2], cs.ap[1]])
                            pv = psa.rearrange("p (k j) -> p k j", k=2)
                            nc.vector.tensor_tensor(pv, cnt_b, pv, op=OP.mult)
                            for k in range(2):
                                h = hp * 2 + k
                                nc.tensor.matmul(
                                    psa[:, k * JB:(k + 1) * JB],
                                    lhsP(h)[:, i0:i0 + 128].bitcast(f32r),
                                    rhsP(h)[:, j0:j0 + JB].bitcast(f32r),
                                    start=False, stop=True,
                                    skip_group_check=True)
                            l_t = mn.tile([128, 2 * JB], f32, tag="l")
                            nc.scalar.activation(l_t, psa, AF.Prelu,
                                                 alpha=NEG_SLOPE)
                            m_t = mpool.tile([128, 2 * JB], f32, tag="m")
                            nc.scalar.activation(m_t, l_t, AF.Exp)
                            mpair.append(m_t)
                        s01 = mn.tile([128, JB], f32, tag="s01")
                        nc.gpsimd.tensor_tensor(s01, mpair[0][:, :JB],
                                                mpair[0][:, JB:], op=OP.add)
                        s23 = mn.tile([128, JB], f32, tag="s23")
                        nc.gpsimd.tensor_tensor(s23, mpair[1][:, :JB],
                                                mpair[1][:, JB:], op=OP.add)
                        s = mn.tile([128, JB], f32, tag="s")
                        nc.vector.tensor_tensor(s, s01, s23, op=OP.add)
                        r = mn.tile([128, JB], f32, tag="r")
                        nc.vector.reciprocal_approx_fast(r, s)
                        rap = r[:, :]
                        r_b = bass.AP(tensor=rap.tensor, offset=rap.offset,
                                      ap=[rap.ap[0], [0, 2], rap.ap[1]])
                        oap = o_t[:, :]
                        for hp in range(H // 2):
                            o_pair = bass.AP(
                                tensor=oap.tensor,
                                offset=oap.offset + (j0 * H + hp * 2),
                                ap=[oap.ap[0], [1, 2], [H, JB]])
                            eng = nc.gpsimd if hp == 1 else nc.vector
                            eng.tensor_tensor(
                                o_pair,
                                mpair[hp].rearrange("p (k j) -> p k j", k=2),
                                r_b, op=OP.mult)
                        nc.sync.dma_start(
                            out_d[i0:i0 + 128, j0 * H:(j0 + JB) * H],
                            o_t[:, j0 * H:(j0 + JB) * H])
    nc.finalize()
    return nc


def kernel(src, edge_index, W_lin, a_src, a_dst, W_edge, a_edge):
    from concourse.bass_utils import run_bass_kernel_spmd

    src = np.ascontiguousarray(np.asarray(src, dtype=np.float32))
    ei = np.asarray(edge_index).astype(np.int64)
    W_lin = np.asarray(W_lin, dtype=np.float32)
    a_src = np.asarray(a_src, dtype=np.float32)
    a_dst = np.asarray(a_dst, dtype=np.float32)
    W_edge = np.asarray(W_edge, dtype=np.float32)
    a_edge = np.asarray(a_edge, dtype=np.float32)

    # fold weights: A = [W_lin@a_src | W_lin@a_dst | W_edge@a_edge]  [128,12]
    A = np.concatenate(
        [W_lin @ a_src, W_lin @ a_dst, W_edge @ a_edge], axis=1
    ).astype(np.float32)
    # edge multiplicity matrix (shared across batches)
    cnt = np.zeros((N, N), np.float32)
    np.add.at(cnt, (ei[0], ei[1]), 1.0)

    nc = _build_nc()
    in_maps = [
        {"src": np.ascontiguousarray(src[b]), "cnt": cnt, "A": A}
        for b in range(B)
    ]
    res = run_bass_kernel_spmd(nc, in_maps, core_ids=list(range(B)))
    out = np.stack(
        [res.results[b]["out"].reshape(N, N, H) for b in range(B)], axis=0
    )
    return out


if __name__ == "__main__":
    rng = np.random.default_rng(0)
    inputs = {
        "src": rng.standard_normal((B, N, F_IN), dtype=np.float32),
        "edge_index": rng.integers(0, N, (2, 32768)).astype(np.int32),
        "W_lin": rng.standard_normal((F_IN, 128), dtype=np.float32) / np.sqrt(F_IN),
        "a_src": rng.standard_normal((128, H), dtype=np.float32) / np.sqrt(128),
        "a_dst": rng.standard_normal((128, H), dtype=np.float32) / np.sqrt(128),
        "W_edge": rng.standard_normal((F_IN, 64), dtype=np.float32) / np.sqrt(F_IN),
        "a_edge": rng.standard_normal((64, H), dtype=np.float32) / np.sqrt(64),
    }
    out = kernel(**inputs)
    print("out", out.shape, out.dtype, out.sum())



# revision 18
# speedup vs baseline: 1.3709x; 1.3709x over previous
"""Trainium2 Bass kernel for nn_AttentionLayer_48722109006175.

Math: out[b,i,j,h] = softmax_h( leaky_relu( s[b,i,h] + d[b,j,h]
                                            + cnt[i,j]*(g[b,i,h]-g[b,j,h]), 0.2 ) )

with s = src@(W_lin@a_src), d = src@(W_lin@a_dst), g = src@(W_edge@a_edge)
and cnt[i,j] the (batch-independent) edge multiplicity matrix.

All three logit contributions accumulate on the PE into one PSUM tile
psa [128, 4*512] (head-blocks of 512 j's), per (i-tile, j-block):
  P:    psa[h] += s_h[i] + d_h[j]          rank-2 f32r matmul (free 512)
  row:  psa[h] += g_h[i]*cnt[i,j]          diag(g_h[i-tile]) @ cnt   (bf16)
  col:  psa[h] -= g_h[j]*cnt[i,j]          (-cnt^T chunk) @ diag(g_h[j-chunk])
All are 1 cycle/row on PE, so the edge scatter costs the same as the
rank-2 part.  The tail is one PSUM->SBUF prelu pass (alternating
ACT/DVE to balance), ACT exp to bf16, pair-tree head sums, custom-DVE
fast reciprocal, and one bf16 2x-mode broadcast multiply.  Output is
stored bf16 in head-plane layout [H,N,N]; the host transposes to
[N,N,H] f32.
Sharding: data-parallel over batch, one batch per NeuronCore.
"""

import numpy as np

B, N, F_IN, H = 8, 1024, 128, 4
JB = 512          # j-block
NT = N // 128     # 8 i-tiles
NC = N // 128     # 8 j-chunks of 128
NEG_SLOPE = 0.2


def _leaky_relu_dve_op():
    """Register (once) a single-input custom-DVE op computing
    out = max(x, NEG_SLOPE*x).  A plain scalar_tensor_tensor(psa, c, psa)
    reads PSUM twice, which the DVE forbids; this op reads Src0 once.
    Registration follows the documented extension path in dve_ops.py
    (append to OPS + the name->row map); the per-NEFF uop table is then
    generated by the normal compile_bir_kernel flow."""
    import numpy as np
    import concourse.dve_ops as dve_ops
    from concourse.dve_spec import Spec, Src0, C2, maxx, lower, _has_src1
    from concourse.dve_uop import DveOpSpec

    NAME = "PRELU_LEAKY_ANT"
    for op in dve_ops.OPS:
        if op.name == NAME:
            return op
    spec = Spec(
        body=maxx(Src0, Src0 * C2),
        reference=lambda in0, in1, s0, s1, imm2: np.maximum(
            in0, in0 * imm2).astype(np.float32),
    )
    row = max(dve_ops._SUB_OPCODE_FOR_NAME.values()) + 1
    assert row < 0x20
    shas = {}
    for ver in ("v3", "v4"):
        compiled = DveOpSpec(name=NAME, opcode=row, uops=lower(spec, ver=ver),
                             rd1_en=_has_src1(spec))
        shas[ver] = compiled.sha(ver)
    op = dve_ops.DveOp(NAME, spec, subdim=False, uops_sha=shas)
    dve_ops.OPS.append(op)
    dve_ops._SUB_OPCODE_FOR_NAME[NAME] = row
    dve_ops.CUSTOM_DVE_SPECS[NAME] = spec
    return op


CFG = {
    "dve_prelu": (1, 3, 5, 7),   # t%9 residues routed to DVE prelu
    "store_per_tile": False,      # (unused in staged pipeline)
    "lp": 4, "mp": 6, "ob": 3, "mn": 4, "cnt": 2,
    "redo0": True,                # recompute i-tile 0 at the end
    "ablate": 0,                  # 0=full .. 6=PE only (debug)
}


def _build_nc():
    import concourse.bass as bass
    import concourse.bacc as bacc
    import concourse.mybir as mybir
    import concourse.tile as tile
    from concourse.masks import make_identity

    prelu_op = _leaky_relu_dve_op()

    f32 = mybir.dt.float32
    f32r = mybir.dt.float32r
    bf16 = mybir.dt.bfloat16
    AF = mybir.ActivationFunctionType
    OP = mybir.AluOpType

    nc = bacc.Bacc()
    # Reset DMA queues + clear bass-managed semaphores at kernel entry.
    # (Bass only emits this when target_bir_lowering=True; without it, stale
    # semaphore/DMA state from previously-executed NEFFs on the same core
    # races the first tile loads.)
    from concourse.bass import compact_to_ranges
    for sem_range in compact_to_ranges(
        [s for s in nc._kernel_sem_range if s not in nc.barrier_sems]
    ):
        nc.gpsimd.dma_reset(sem_range)
        nc.gpsimd.sem_clear(sem_range)
    nc._nrt_pseudo_barrier()

    src_d = nc.dram_tensor("src", [N, F_IN], f32, kind="ExternalInput")
    # rows 0..N: cnt (bf16); rows N..2N: -cnt^T packed per-i-tile slab
    cn_d = nc.dram_tensor("cn", [2 * N, N], bf16, kind="ExternalInput")
    a_d = nc.dram_tensor("A", [F_IN, 12], f32, kind="ExternalInput")
    out_d = nc.dram_tensor("out", [H, N, N], bf16, kind="ExternalOutput")

    with tile.TileContext(nc) as tc:
        with tc.tile_pool(name="stage", bufs=1) as stage:
            # P-matmul staging: lhsAll row0 = [s_0..s_3] chunks, row1 = ones;
            # rhsAll row0 = ones, row1 = [d_0..d_3] chunks.
            lhsAll = stage.tile([2, H * N], f32)
            rhsAll = stage.tile([2, H * N], f32)
            # diag(g_h[chunk c]) tiles: dp[c][:, h*128:(h+1)*128], bf16
            dps = [stage.tile([128, H * 128], bf16, name=f"dp{c}")
                   for c in range(NC)]
            sdg_sb = stage.tile([128, NT * 12], f32)   # sdg chunks, [i, c*12+k]

            # ---- prologue ----
            with tc.tile_pool(name="pro", bufs=1) as pro, \
                 tc.tile_pool(name="pps", bufs=2, space="PSUM") as pps:
                ones_t = pro.tile([1, N], f32)
                nc.vector.memset(ones_t, 1.0)
                oap1 = ones_t[0:1, :]
                ones_b = bass.AP(tensor=oap1.tensor, offset=oap1.offset,
                                 ap=[oap1.ap[0], [0, H], oap1.ap[1]])
                nc.sync.dma_start(lhsAll[1:2, :], ones_b)
                nc.sync.dma_start(rhsAll[0:1, :], ones_b)
                ident = pro.tile([128, 128], f32)
                make_identity(nc, ident)
                ident_bf = pro.tile([128, 128], bf16)
                nc.vector.tensor_scalar(ident_bf, ident, 1.0, None, op0=OP.mult)
                a_sb = pro.tile([F_IN, 12], f32)
                nc.gpsimd.dma_start(a_sb, a_d[:, :])
                chunks = pro.tile([128, N], f32)
                nc.sync.dma_start(
                    chunks.rearrange("p (c f) -> p c f", c=NT),
                    src_d[:, :].rearrange("(c p) f -> p c f", p=128))
                srcT = pro.tile([128, N], f32)
                sdgT = pro.tile([12, N], f32)
                cview = chunks.rearrange("p (c f) -> p c f", c=NT)
                for c in range(NT):
                    pt = pps.tile([128, 128], f32, tag="pt")
                    nc.tensor.transpose(pt, cview[:, c, :], ident)
                    nc.scalar.copy(srcT[:, c * 128:(c + 1) * 128], pt)
                for half in range(2):
                    ps = pps.tile([12, 512], f32, tag="sdg")
                    nc.tensor.matmul(ps, a_sb,
                                     srcT[:, half * 512:(half + 1) * 512],
                                     start=True, stop=True)
                    nc.scalar.copy(sdgT[:, half * 512:(half + 1) * 512], ps)
                # sdg (non-transposed) chunks for per-partition g columns
                for c in range(NT):
                    ps12 = pps.tile([128, 12], f32, tag="sdgc")
                    nc.tensor.matmul(ps12, srcT[:, c * 128:(c + 1) * 128],
                                     a_sb, start=True, stop=True)
                    nc.scalar.copy(sdg_sb[:, c * 12:(c + 1) * 12], ps12)
                # staging rows: s chunks + d chunks
                nc.sync.dma_start(lhsAll[0:1, :], sdgT[0:4, :])
                nc.sync.dma_start(rhsAll[1:2, :], sdgT[4:8, :])
                # diag tiles: dp[c][:, h*128:(h+1)*128] = ident * g_h[c-chunk]
                for c in range(NC):
                    for h in range(H):
                        g_col = sdg_sb[:, c * 12 + 8 + h: c * 12 + 9 + h]
                        nc.vector.tensor_scalar(
                            dps[c][:, h * 128:(h + 1) * 128],
                            ident_bf, g_col, None, op0=OP.mult)

            def lhsP(h):
                return lhsAll[:, h * N:(h + 1) * N]

            def rhsP(h):
                return rhsAll[:, h * N:(h + 1) * N]

            # ---- main loop ----
            with tc.tile_pool(name="mn", bufs=CFG["mn"]) as mn, \
                 tc.tile_pool(name="lp", bufs=CFG["lp"]) as lp, \
                 tc.tile_pool(name="mp", bufs=CFG["mp"]) as mp, \
                 tc.tile_pool(name="ob", bufs=CFG["ob"]) as obp, \
                 tc.tile_pool(name="cntp", bufs=CFG["cnt"]) as cntp, \
                 tc.tile_pool(name="ps", bufs=2, space="PSUM") as psp:
                # recompute i-tile 0 last: its startup-issued version can race
                # semaphore warm-up; the final DRAM write wins.
                order = list(range(NT)) + ([0] if CFG["redo0"] else [])
                cnt_pref = {}

                def load_cnt(idx):
                    it = order[idx]
                    t = cntp.tile([128, 2 * N], bf16, tag="cnt",
                                  name=f"cnt{idx}")
                    cnap = cn_d[:, :]
                    src_ap = bass.AP(
                        tensor=cnap.tensor, offset=it * 128 * N,
                        ap=[[N, 128], [N * N, 2], [1, N]])
                    nc.sync.dma_start(
                        t.rearrange("p (b j) -> p b j", b=2), src_ap)
                    return t

                cnt_pref[0] = load_cnt(0)
                if len(order) > 1:
                    cnt_pref[1] = load_cnt(1)
                tiles = [(idx, it, jb) for idx, it in enumerate(order)
                         for jb in range(N // JB)]

                # Software pipeline: stage A at tile t, B at t-1, C at t-2,
                # D at t-3.  Every instruction an engine dequeues has inputs
                # that finished >= 1 tile ago, so the in-order engine queues
                # never head-of-line block on cross-engine round trips.
                state = {}

                def stage_a(t):
                    idx, it, jb = tiles[t]
                    i0, j0 = it * 128, jb * JB
                    if jb == 0:
                        cn_t = cnt_pref.pop(idx)
                        if idx + 2 < len(order):
                            cnt_pref[idx + 2] = load_cnt(idx + 2)
                        state[("cn", idx)] = cn_t
                    cn_t = state[("cn", idx)]
                    cnt_t = cn_t[:, 0:N]
                    nct_t = cn_t[:, N:2 * N]
                    psa = psp.tile([128, H * JB], f32, tag="psa")
                    for h in range(H):
                        nc.tensor.matmul(
                            psa[:, h * JB:(h + 1) * JB],
                            lhsP(h)[:, i0:i0 + 128].bitcast(f32r),
                            rhsP(h)[:, j0:j0 + JB].bitcast(f32r),
                            start=True, stop=True)
                    for h in range(H):
                        nc.tensor.matmul(
                            psa[:, h * JB:(h + 1) * JB],
                            dps[it][:, h * 128:(h + 1) * 128],
                            cnt_t[:, j0:j0 + JB],
                            start=False, stop=True, skip_group_check=True)
                    pv = psa.rearrange("p (h j) -> p h j", h=H)
                    for c in range(JB // 128):
                        jc = jb * (JB // 128) + c
                        nc.tensor.matmul(
                            pv[:, :, c * 128:(c + 1) * 128],
                            nct_t[:, jc * 128:(jc + 1) * 128],
                            dps[jc],
                            start=False, stop=True,
                            skip_group_check=True)
                    if CFG["ablate"] >= 6:
                        return
                    l_t = lp.tile([128, H * JB], f32, tag="l")
                    if (t % 9) in CFG["dve_prelu"]:
                        nc.vector._custom_dve(
                            prelu_op, out=l_t, in0=psa, imm2=NEG_SLOPE)
                    else:
                        nc.scalar.activation(l_t, psa, AF.Prelu,
                                             alpha=NEG_SLOPE)
                    state[("l", t)] = l_t

                def stage_b(t):
                    if CFG["ablate"] >= 5:
                        return
                    l_t = state.pop(("l", t))
                    m_t = mp.tile([128, H * JB], bf16, tag="m")
                    nc.scalar.activation(m_t, l_t, AF.Exp)
                    state[("m", t)] = m_t
                    if CFG["ablate"] >= 4:
                        return
                    s2 = mn.tile([128, 2 * JB], bf16, tag="s2")
                    nc.vector.tensor_tensor(
                        s2, m_t[:, 0:2 * JB], m_t[:, 2 * JB:4 * JB],
                        op=OP.add)
                    s_f = mn.tile([128, JB], f32, tag="s")
                    nc.gpsimd.tensor_tensor(
                        s_f, s2[:, 0:JB], s2[:, JB:2 * JB], op=OP.add)
                    state[("s", t)] = s_f

                def stage_c(t):
                    if CFG["ablate"] >= 4:
                        return
                    s_f = state.pop(("s", t))
                    if CFG["ablate"] >= 3:
                        return
                    r_f = mn.tile([128, JB], f32, tag="r")
                    nc.vector.reciprocal_approx_fast(r_f, s_f)
                    r_b = mn.tile([128, JB], bf16, tag="rb")
                    nc.gpsimd.tensor_scalar(r_b, r_f, 1.0, None, op0=OP.mult)
                    state[("r", t)] = r_b

                def stage_d(t):
                    if CFG["ablate"] >= 3:
                        return
                    idx, it, jb = tiles[t]
                    i0, j0 = it * 128, jb * JB
                    m_t = state.pop(("m", t))
                    r_b = state.pop(("r", t))
                    rap = r_b[:, :]
                    r_b4 = bass.AP(tensor=rap.tensor, offset=rap.offset,
                                   ap=[rap.ap[0], [0, H], rap.ap[1]])
                    o_t = obp.tile([128, H * JB], bf16, tag="o")
                    nc.vector.tensor_tensor(
                        o_t.rearrange("p (h j) -> p h j", h=H),
                        m_t.rearrange("p (h j) -> p h j", h=H),
                        r_b4, op=OP.mult)
                    if CFG["ablate"] >= 1:
                        return
                    nc.sync.dma_start(
                        out_d[:, i0:i0 + 128, j0:j0 + JB]
                        .rearrange("h p j -> p h j"),
                        o_t.rearrange("p (h j) -> p h j", h=H))

                nt_total = len(tiles)
                stage_order = CFG.get("stage_order", "abcd")
                fns = {"a": (stage_a, 0), "b": (stage_b, 1),
                       "c": (stage_c, 2), "d": (stage_d, 3)}
                for t in range(nt_total + 3):
                    for ch in stage_order:
                        fn, lag = fns[ch]
                        if 0 <= t - lag < nt_total:
                            fn(t - lag)
    nc.finalize()
    return nc


def _prepare_in_maps(src, edge_index, W_lin, a_src, a_dst, W_edge, a_edge):
    import ml_dtypes

    src = np.ascontiguousarray(np.asarray(src, dtype=np.float32))
    ei = np.asarray(edge_index).astype(np.int64)
    W_lin = np.asarray(W_lin, dtype=np.float32)
    a_src = np.asarray(a_src, dtype=np.float32)
    a_dst = np.asarray(a_dst, dtype=np.float32)
    W_edge = np.asarray(W_edge, dtype=np.float32)
    a_edge = np.asarray(a_edge, dtype=np.float32)

    # fold weights: A = [W_lin@a_src | W_lin@a_dst | W_edge@a_edge]  [128,12]
    A = np.concatenate(
        [W_lin @ a_src, W_lin @ a_dst, W_edge @ a_edge], axis=1
    ).astype(np.float32)
    # edge multiplicity matrix (shared across batches)
    cnt = np.zeros((N, N), np.float32)
    np.add.at(cnt, (ei[0], ei[1]), 1.0)
    # -cnt^T packed per-i-tile: ncntT[it*128+p, q*128+f] = -cnt[it*128+f, q*128+p]
    T = np.ascontiguousarray((-cnt).T)          # T[j, i] = -cnt[i, j]
    ncntT = T.reshape(NT, 128, NT, 128).transpose(2, 1, 0, 3).reshape(N, N)
    cn = np.ascontiguousarray(
        np.concatenate([cnt, ncntT], axis=0)).astype(ml_dtypes.bfloat16)
    return [
        {"src": np.ascontiguousarray(src[b]), "cn": cn, "A": A}
        for b in range(B)
    ]


def kernel(src, edge_index, W_lin, a_src, a_dst, W_edge, a_edge):
    from concourse.bass_utils import run_bass_kernel_spmd

    in_maps = _prepare_in_maps(src, edge_index, W_lin, a_src, a_dst,
                               W_edge, a_edge)
    nc = _build_nc()
    res = run_bass_kernel_spmd(nc, in_maps, core_ids=list(range(B)))
    out = np.stack(
        [np.asarray(res.results[b]["out"]).astype(np.float32)
         .transpose(1, 2, 0) for b in range(B)], axis=0)
    return np.ascontiguousarray(out)


if __name__ == "__main__":
    rng = np.random.default_rng(0)
    inputs = {
        "src": rng.standard_normal((B, N, F_IN), dtype=np.float32),
        "edge_index": rng.integers(0, N, (2, 32768)).astype(np.int32),
        "W_lin": rng.standard_normal((F_IN, 128), dtype=np.float32) / np.sqrt(F_IN),
        "a_src": rng.standard_normal((128, H), dtype=np.float32) / np.sqrt(128),
        "a_dst": rng.standard_normal((128, H), dtype=np.float32) / np.sqrt(128),
        "W_edge": rng.standard_normal((F_IN, 64), dtype=np.float32) / np.sqrt(F_IN),
        "a_edge": rng.standard_normal((64, H), dtype=np.float32) / np.sqrt(64),
    }
    out = kernel(**inputs)
    print("out", out.shape, out.dtype, out.sum())


# revision 19
# speedup vs baseline: 1.4829x; 1.0818x over previous
"""Trainium2 Bass kernel for nn_AttentionLayer_48722109006175.

Math: out[b,i,j,h] = softmax_h( leaky_relu( s[b,i,h] + d[b,j,h]
                                            + cnt[i,j]*(g[b,i,h]-g[b,j,h]), 0.2 ) )

with s = src@(W_lin@a_src), d = src@(W_lin@a_dst), g = src@(W_edge@a_edge)
and cnt[i,j] the (batch-independent) edge multiplicity matrix.

All three logit contributions accumulate on the PE into one PSUM tile
psa [128, 4*512] (head-blocks of 512 j's), per (i-tile, j-block):
  P:    psa[h] += s_h[i] + d_h[j]          rank-2 f32r matmul (free 512)
  row:  psa[h] += g_h[i]*cnt[i,j]          diag(g_h[i-tile]) @ cnt   (bf16)
  col:  psa[h] -= g_h[j]*cnt[i,j]          (-cnt^T chunk) @ diag(g_h[j-chunk])
All are 1 cycle/row on PE, so the edge scatter costs the same as the
rank-2 part.  The tail is one PSUM->SBUF prelu pass (alternating
ACT/DVE to balance), ACT exp to bf16, pair-tree head sums, custom-DVE
fast reciprocal, and one bf16 2x-mode broadcast multiply.  Output is
stored bf16 in head-plane layout [H,N,N]; the host transposes to
[N,N,H] f32.
Sharding: data-parallel over batch, one batch per NeuronCore.
"""

import numpy as np

B, N, F_IN, H = 8, 1024, 128, 4
JB = 512          # j-block
NT = N // 128     # 8 i-tiles
NC = N // 128     # 8 j-chunks of 128
NEG_SLOPE = 0.2


def _leaky_relu_dve_op():
    """Register (once) a single-input custom-DVE op computing
    out = max(x, NEG_SLOPE*x).  A plain scalar_tensor_tensor(psa, c, psa)
    reads PSUM twice, which the DVE forbids; this op reads Src0 once.
    Registration follows the documented extension path in dve_ops.py
    (append to OPS + the name->row map); the per-NEFF uop table is then
    generated by the normal compile_bir_kernel flow."""
    import numpy as np
    import concourse.dve_ops as dve_ops
    from concourse.dve_spec import Spec, Src0, C2, maxx, lower, _has_src1
    from concourse.dve_uop import DveOpSpec

    NAME = "PRELU_LEAKY_ANT"
    for op in dve_ops.OPS:
        if op.name == NAME:
            return op
    spec = Spec(
        body=maxx(Src0, Src0 * C2),
        reference=lambda in0, in1, s0, s1, imm2: np.maximum(
            in0, in0 * imm2).astype(np.float32),
    )
    row = max(dve_ops._SUB_OPCODE_FOR_NAME.values()) + 1
    assert row < 0x20
    shas = {}
    for ver in ("v3", "v4"):
        compiled = DveOpSpec(name=NAME, opcode=row, uops=lower(spec, ver=ver),
                             rd1_en=_has_src1(spec))
        shas[ver] = compiled.sha(ver)
    op = dve_ops.DveOp(NAME, spec, subdim=False, uops_sha=shas)
    dve_ops.OPS.append(op)
    dve_ops._SUB_OPCODE_FOR_NAME[NAME] = row
    dve_ops.CUSTOM_DVE_SPECS[NAME] = spec
    return op


CFG = {
    "dve_prelu": (1, 3, 5, 7),   # t%9 residues routed to DVE prelu
    "store_per_tile": False,      # (unused in staged pipeline)
    "lp": 4, "mp": 6, "ob": 3, "mn": 4, "cnt": 2,
    "redo0": False,               # recompute i-tile 0 at the end
    "ablate": 0,                  # 0=full .. 6=PE only (debug)
    "stage_order": "bacd",
}


def _build_nc():
    import concourse.bass as bass
    import concourse.bacc as bacc
    import concourse.mybir as mybir
    import concourse.tile as tile
    from concourse.masks import make_identity

    prelu_op = _leaky_relu_dve_op()

    f32 = mybir.dt.float32
    f32r = mybir.dt.float32r
    bf16 = mybir.dt.bfloat16
    AF = mybir.ActivationFunctionType
    OP = mybir.AluOpType

    nc = bacc.Bacc()
    # Reset DMA queues + clear bass-managed semaphores at kernel entry.
    # (Bass only emits this when target_bir_lowering=True; without it, stale
    # semaphore/DMA state from previously-executed NEFFs on the same core
    # races the first tile loads.)
    from concourse.bass import compact_to_ranges
    for sem_range in compact_to_ranges(
        [s for s in nc._kernel_sem_range if s not in nc.barrier_sems]
    ):
        nc.gpsimd.dma_reset(sem_range)
        nc.gpsimd.sem_clear(sem_range)
    nc._nrt_pseudo_barrier()

    src_d = nc.dram_tensor("src", [N, F_IN], f32, kind="ExternalInput")
    # rows 0..N: cnt (bf16); rows N..2N: -cnt^T packed per-i-tile slab
    cn_d = nc.dram_tensor("cn", [2 * N, N], bf16, kind="ExternalInput")
    a_d = nc.dram_tensor("A", [F_IN, 12], f32, kind="ExternalInput")
    out_d = nc.dram_tensor("out", [H, N, N], bf16, kind="ExternalOutput")

    with tile.TileContext(nc) as tc:
        with tc.tile_pool(name="stage", bufs=1) as stage:
            # P-matmul staging: lhsAll row0 = [s_0..s_3] chunks, row1 = ones;
            # rhsAll row0 = ones, row1 = [d_0..d_3] chunks.
            lhsAll = stage.tile([2, H * N], f32)
            rhsAll = stage.tile([2, H * N], f32)
            # diag(g_h[chunk c]) tiles: dp[c][:, h*128:(h+1)*128], bf16
            dps = [stage.tile([128, H * 128], bf16, name=f"dp{c}")
                   for c in range(NC)]
            sdg_sb = stage.tile([128, NT * 12], f32)   # sdg chunks, [i, c*12+k]

            # ---- prologue ----
            with tc.tile_pool(name="pro", bufs=1) as pro, \
                 tc.tile_pool(name="pps", bufs=2, space="PSUM") as pps:
                ones_t = pro.tile([1, N], f32)
                nc.vector.memset(ones_t, 1.0)
                oap1 = ones_t[0:1, :]
                ones_b = bass.AP(tensor=oap1.tensor, offset=oap1.offset,
                                 ap=[oap1.ap[0], [0, H], oap1.ap[1]])
                nc.sync.dma_start(lhsAll[1:2, :], ones_b)
                nc.sync.dma_start(rhsAll[0:1, :], ones_b)
                ident = pro.tile([128, 128], f32)
                make_identity(nc, ident)
                ident_bf = pro.tile([128, 128], bf16)
                nc.vector.tensor_scalar(ident_bf, ident, 1.0, None, op0=OP.mult)
                a_sb = pro.tile([F_IN, 12], f32)
                nc.gpsimd.dma_start(a_sb, a_d[:, :])
                chunks = pro.tile([128, N], f32)
                nc.sync.dma_start(
                    chunks.rearrange("p (c f) -> p c f", c=NT),
                    src_d[:, :].rearrange("(c p) f -> p c f", p=128))
                srcT = pro.tile([128, N], f32)
                sdgT = pro.tile([12, N], f32)
                cview = chunks.rearrange("p (c f) -> p c f", c=NT)
                for c in range(NT):
                    pt = pps.tile([128, 128], f32, tag="pt")
                    nc.tensor.transpose(pt, cview[:, c, :], ident)
                    nc.scalar.copy(srcT[:, c * 128:(c + 1) * 128], pt)
                for half in range(2):
                    ps = pps.tile([12, 512], f32, tag="sdg")
                    nc.tensor.matmul(ps, a_sb,
                                     srcT[:, half * 512:(half + 1) * 512],
                                     start=True, stop=True)
                    nc.scalar.copy(sdgT[:, half * 512:(half + 1) * 512], ps)
                # sdg (non-transposed) chunks for per-partition g columns
                for c in range(NT):
                    ps12 = pps.tile([128, 12], f32, tag="sdgc")
                    nc.tensor.matmul(ps12, srcT[:, c * 128:(c + 1) * 128],
                                     a_sb, start=True, stop=True)
                    nc.scalar.copy(sdg_sb[:, c * 12:(c + 1) * 12], ps12)
                # staging rows: s chunks + d chunks
                nc.sync.dma_start(lhsAll[0:1, :], sdgT[0:4, :])
                nc.sync.dma_start(rhsAll[1:2, :], sdgT[4:8, :])
                # diag tiles: dp[c][:, h*128:(h+1)*128] = ident * g_h[c-chunk]
                for c in range(NC):
                    for h in range(H):
                        g_col = sdg_sb[:, c * 12 + 8 + h: c * 12 + 9 + h]
                        nc.vector.tensor_scalar(
                            dps[c][:, h * 128:(h + 1) * 128],
                            ident_bf, g_col, None, op0=OP.mult)

            def lhsP(h):
                return lhsAll[:, h * N:(h + 1) * N]

            def rhsP(h):
                return rhsAll[:, h * N:(h + 1) * N]

            # ---- main loop ----
            with tc.tile_pool(name="mn", bufs=CFG["mn"]) as mn, \
                 tc.tile_pool(name="lp", bufs=CFG["lp"]) as lp, \
                 tc.tile_pool(name="mp", bufs=CFG["mp"]) as mp, \
                 tc.tile_pool(name="ob", bufs=CFG["ob"]) as obp, \
                 tc.tile_pool(name="cntp", bufs=CFG["cnt"]) as cntp, \
                 tc.tile_pool(name="ps", bufs=2, space="PSUM") as psp:
                # recompute i-tile 0 last: its startup-issued version can race
                # semaphore warm-up; the final DRAM write wins.
                order = list(range(NT)) + ([0] if CFG["redo0"] else [])
                cnt_pref = {}

                def load_cnt(idx):
                    it = order[idx]
                    t = cntp.tile([128, 2 * N], bf16, tag="cnt",
                                  name=f"cnt{idx}")
                    cnap = cn_d[:, :]
                    src_ap = bass.AP(
                        tensor=cnap.tensor, offset=it * 128 * N,
                        ap=[[N, 128], [N * N, 2], [1, N]])
                    nc.sync.dma_start(
                        t.rearrange("p (b j) -> p b j", b=2), src_ap)
                    return t

                cnt_pref[0] = load_cnt(0)
                if len(order) > 1:
                    cnt_pref[1] = load_cnt(1)
                tiles = [(idx, it, jb) for idx, it in enumerate(order)
                         for jb in range(N // JB)]

                # Software pipeline: stage A at tile t, B at t-1, C at t-2,
                # D at t-3.  Every instruction an engine dequeues has inputs
                # that finished >= 1 tile ago, so the in-order engine queues
                # never head-of-line block on cross-engine round trips.
                state = {}

                def stage_a(t):
                    idx, it, jb = tiles[t]
                    i0, j0 = it * 128, jb * JB
                    if jb == 0:
                        cn_t = cnt_pref.pop(idx)
                        if idx + 2 < len(order):
                            cnt_pref[idx + 2] = load_cnt(idx + 2)
                        state[("cn", idx)] = cn_t
                    cn_t = state[("cn", idx)]
                    cnt_t = cn_t[:, 0:N]
                    nct_t = cn_t[:, N:2 * N]
                    psa = psp.tile([128, H * JB], f32, tag="psa")
                    for h in range(H):
                        nc.tensor.matmul(
                            psa[:, h * JB:(h + 1) * JB],
                            lhsP(h)[:, i0:i0 + 128].bitcast(f32r),
                            rhsP(h)[:, j0:j0 + JB].bitcast(f32r),
                            start=True, stop=True)
                    for h in range(H):
                        nc.tensor.matmul(
                            psa[:, h * JB:(h + 1) * JB],
                            dps[it][:, h * 128:(h + 1) * 128],
                            cnt_t[:, j0:j0 + JB],
                            start=False, stop=True, skip_group_check=True)
                    pv = psa.rearrange("p (h j) -> p h j", h=H)
                    for c in range(JB // 128):
                        jc = jb * (JB // 128) + c
                        nc.tensor.matmul(
                            pv[:, :, c * 128:(c + 1) * 128],
                            nct_t[:, jc * 128:(jc + 1) * 128],
                            dps[jc],
                            start=False, stop=True,
                            skip_group_check=True)
                    if CFG["ablate"] >= 6:
                        return
                    l_t = lp.tile([128, H * JB], f32, tag="l")
                    if (t % 9) in CFG["dve_prelu"]:
                        nc.vector._custom_dve(
                            prelu_op, out=l_t, in0=psa, imm2=NEG_SLOPE)
                    else:
                        nc.scalar.activation(l_t, psa, AF.Prelu,
                                             alpha=NEG_SLOPE)
                    state[("l", t)] = l_t

                def stage_b(t):
                    if CFG["ablate"] >= 5:
                        return
                    l_t = state.pop(("l", t))
                    m_t = mp.tile([128, H * JB], bf16, tag="m")
                    nc.scalar.activation(m_t, l_t, AF.Exp)
                    state[("m", t)] = m_t
                    if CFG["ablate"] >= 4:
                        return
                    s2 = mn.tile([128, 2 * JB], bf16, tag="s2")
                    nc.vector.tensor_tensor(
                        s2, m_t[:, 0:2 * JB], m_t[:, 2 * JB:4 * JB],
                        op=OP.add)
                    s_f = mn.tile([128, JB], f32, tag="s")
                    nc.gpsimd.tensor_tensor(
                        s_f, s2[:, 0:JB], s2[:, JB:2 * JB], op=OP.add)
                    state[("s", t)] = s_f

                def stage_c(t):
                    if CFG["ablate"] >= 4:
                        return
                    s_f = state.pop(("s", t))
                    if CFG["ablate"] >= 3:
                        return
                    r_f = mn.tile([128, JB], f32, tag="r")
                    nc.vector.reciprocal_approx_fast(r_f, s_f)
                    r_b = mn.tile([128, JB], bf16, tag="rb")
                    nc.gpsimd.tensor_scalar(r_b, r_f, 1.0, None, op0=OP.mult)
                    state[("r", t)] = r_b

                def stage_d(t):
                    if CFG["ablate"] >= 3:
                        return
                    idx, it, jb = tiles[t]
                    i0, j0 = it * 128, jb * JB
                    m_t = state.pop(("m", t))
                    r_b = state.pop(("r", t))
                    rap = r_b[:, :]
                    r_b4 = bass.AP(tensor=rap.tensor, offset=rap.offset,
                                   ap=[rap.ap[0], [0, H], rap.ap[1]])
                    o_t = obp.tile([128, H * JB], bf16, tag="o")
                    nc.vector.tensor_tensor(
                        o_t.rearrange("p (h j) -> p h j", h=H),
                        m_t.rearrange("p (h j) -> p h j", h=H),
                        r_b4, op=OP.mult)
                    if CFG["ablate"] >= 1:
                        return
                    nc.sync.dma_start(
                        out_d[:, i0:i0 + 128, j0:j0 + JB]
                        .rearrange("h p j -> p h j"),
                        o_t.rearrange("p (h j) -> p h j", h=H))

                nt_total = len(tiles)
                stage_order = CFG.get("stage_order", "abcd")
                fns = {"a": (stage_a, 0), "b": (stage_b, 1),
                       "c": (stage_c, 2), "d": (stage_d, 3)}
                for t in range(nt_total + 3):
                    for ch in stage_order:
                        fn, lag = fns[ch]
                        if 0 <= t - lag < nt_total:
                            fn(t - lag)
    nc.finalize()
    return nc


def _prepare_in_maps(src, edge_index, W_lin, a_src, a_dst, W_edge, a_edge):
    import ml_dtypes

    src = np.ascontiguousarray(np.asarray(src, dtype=np.float32))
    ei = np.asarray(edge_index).astype(np.int64)
    W_lin = np.asarray(W_lin, dtype=np.float32)
    a_src = np.asarray(a_src, dtype=np.float32)
    a_dst = np.asarray(a_dst, dtype=np.float32)
    W_edge = np.asarray(W_edge, dtype=np.float32)
    a_edge = np.asarray(a_edge, dtype=np.float32)

    # fold weights: A = [W_lin@a_src | W_lin@a_dst | W_edge@a_edge]  [128,12]
    A = np.concatenate(
        [W_lin @ a_src, W_lin @ a_dst, W_edge @ a_edge], axis=1
    ).astype(np.float32)
    # edge multiplicity matrix (shared across batches)
    cnt = np.zeros((N, N), np.float32)
    np.add.at(cnt, (ei[0], ei[1]), 1.0)
    # -cnt^T packed per-i-tile: ncntT[it*128+p, q*128+f] = -cnt[it*128+f, q*128+p]
    T = np.ascontiguousarray((-cnt).T)          # T[j, i] = -cnt[i, j]
    ncntT = T.reshape(NT, 128, NT, 128).transpose(2, 1, 0, 3).reshape(N, N)
    cn = np.ascontiguousarray(
        np.concatenate([cnt, ncntT], axis=0)).astype(ml_dtypes.bfloat16)
    return [
        {"src": np.ascontiguousarray(src[b]), "cn": cn, "A": A}
        for b in range(B)
    ]


def kernel(src, edge_index, W_lin, a_src, a_dst, W_edge, a_edge):
    from concourse.bass_utils import run_bass_kernel_spmd

    in_maps = _prepare_in_maps(src, edge_index, W_lin, a_src, a_dst,
                               W_edge, a_edge)
    nc = _build_nc()
    res = run_bass_kernel_spmd(nc, in_maps, core_ids=list(range(B)))
    out = np.stack(
        [np.asarray(res.results[b]["out"]).astype(np.float32)
         .transpose(1, 2, 0) for b in range(B)], axis=0)
    return np.ascontiguousarray(out)


if __name__ == "__main__":
    rng = np.random.default_rng(0)
    inputs = {
        "src": rng.standard_normal((B, N, F_IN), dtype=np.float32),
        "edge_index": rng.integers(0, N, (2, 32768)).astype(np.int32),
        "W_lin": rng.standard_normal((F_IN, 128), dtype=np.float32) / np.sqrt(F_IN),
        "a_src": rng.standard_normal((128, H), dtype=np.float32) / np.sqrt(128),
        "a_dst": rng.standard_normal((128, H), dtype=np.float32) / np.sqrt(128),
        "W_edge": rng.standard_normal((F_IN, 64), dtype=np.float32) / np.sqrt(F_IN),
        "a_edge": rng.standard_normal((64, H), dtype=np.float32) / np.sqrt(64),
    }
    out = kernel(**inputs)
    print("out", out.shape, out.dtype, out.sum())


# revision 41
# speedup vs baseline: 1.4903x; 1.0049x over previous
"""Trainium2 Bass kernel for nn_AttentionLayer_48722109006175.

Math: out[b,i,j,h] = softmax_h( leaky_relu( s[b,i,h] + d[b,j,h]
                                            + cnt[i,j]*(g[b,i,h]-g[b,j,h]), 0.2 ) )

with s = src@(W_lin@a_src), d = src@(W_lin@a_dst), g = src@(W_edge@a_edge)
and cnt[i,j] the (batch-independent) edge multiplicity matrix.

All three logit contributions accumulate on the PE into one PSUM tile
psa [128, 4*512] (head-blocks of 512 j's), per (i-tile, j-block):
  P:    psa[h] += s_h[i] + d_h[j]          rank-2 f32r matmul (free 512)
  row:  psa[h] += g_h[i]*cnt[i,j]          diag(g_h[i-tile]) @ cnt   (bf16)
  col:  psa[h] -= g_h[j]*cnt[i,j]          (-cnt^T chunk) @ diag(g_h[j-chunk])
All are 1 cycle/row on PE, so the edge scatter costs the same as the
rank-2 part.  The tail is one PSUM->SBUF prelu pass (alternating
ACT/DVE to balance), ACT exp to bf16, pair-tree head sums, custom-DVE
fast reciprocal, and one bf16 2x-mode broadcast multiply.  Output is
stored bf16 in head-plane layout [H,N,N]; the host transposes to
[N,N,H] f32.
Sharding: data-parallel over batch, one batch per NeuronCore.
"""

import numpy as np

B, N, F_IN, H = 8, 1024, 128, 4
JB = 512          # j-block
NT = N // 128     # 8 i-tiles
NC = N // 128     # 8 j-chunks of 128
NEG_SLOPE = 0.2


def _leaky_relu_dve_op():
    """Register (once) a single-input custom-DVE op computing
    out = max(x, NEG_SLOPE*x).  A plain scalar_tensor_tensor(psa, c, psa)
    reads PSUM twice, which the DVE forbids; this op reads Src0 once.
    Registration follows the documented extension path in dve_ops.py
    (append to OPS + the name->row map); the per-NEFF uop table is then
    generated by the normal compile_bir_kernel flow."""
    import numpy as np
    import concourse.dve_ops as dve_ops
    from concourse.dve_spec import Spec, Src0, C2, maxx, lower, _has_src1
    from concourse.dve_uop import DveOpSpec

    NAME = "PRELU_LEAKY_ANT"
    for op in dve_ops.OPS:
        if op.name == NAME:
            return op
    spec = Spec(
        body=maxx(Src0, Src0 * C2),
        reference=lambda in0, in1, s0, s1, imm2: np.maximum(
            in0, in0 * imm2).astype(np.float32),
    )
    row = max(dve_ops._SUB_OPCODE_FOR_NAME.values()) + 1
    assert row < 0x20
    shas = {}
    for ver in ("v3", "v4"):
        compiled = DveOpSpec(name=NAME, opcode=row, uops=lower(spec, ver=ver),
                             rd1_en=_has_src1(spec))
        shas[ver] = compiled.sha(ver)
    op = dve_ops.DveOp(NAME, spec, subdim=False, uops_sha=shas)
    dve_ops.OPS.append(op)
    dve_ops._SUB_OPCODE_FOR_NAME[NAME] = row
    dve_ops.CUSTOM_DVE_SPECS[NAME] = spec
    return op


CFG = {
    "dve_prelu": (1, 3, 5, 7),   # t%9 residues routed to DVE prelu
    "store_per_tile": False,      # (unused in staged pipeline)
    "lp": 4, "mp": 7, "ob": 3, "mn": 5, "cnt": 2,
    "redo0": False,               # recompute i-tile 0 at the end
    "ablate": 0,                  # 0=full .. 6=PE only (debug)
    "stage_order": "abecd",
    "divide": False,
    "pool_s2": (),
    "pool_mult": 0,
}


def _build_nc():
    import concourse.bass as bass
    import concourse.bacc as bacc
    import concourse.mybir as mybir
    import concourse.tile as tile
    from concourse.masks import make_identity

    prelu_op = _leaky_relu_dve_op()

    f32 = mybir.dt.float32
    f32r = mybir.dt.float32r
    bf16 = mybir.dt.bfloat16
    AF = mybir.ActivationFunctionType
    OP = mybir.AluOpType

    nc = bacc.Bacc()
    # Reset DMA queues + clear bass-managed semaphores at kernel entry.
    # (Bass only emits this when target_bir_lowering=True; without it, stale
    # semaphore/DMA state from previously-executed NEFFs on the same core
    # races the first tile loads.)
    from concourse.bass import compact_to_ranges
    for sem_range in compact_to_ranges(
        [s for s in nc._kernel_sem_range if s not in nc.barrier_sems]
    ):
        nc.gpsimd.dma_reset(sem_range)
        nc.gpsimd.sem_clear(sem_range)
    nc._nrt_pseudo_barrier()

    src_d = nc.dram_tensor("src", [N, F_IN], f32, kind="ExternalInput")
    # rows 0..N: cnt (bf16); rows N..2N: -cnt^T packed per-i-tile slab
    cn_d = nc.dram_tensor("cn", [2 * N, N], bf16, kind="ExternalInput")
    a_d = nc.dram_tensor("A", [F_IN, 12], f32, kind="ExternalInput")
    out_d = nc.dram_tensor("out", [H, N, N], bf16, kind="ExternalOutput")

    with tile.TileContext(nc) as tc:
        with tc.tile_pool(name="stage", bufs=1) as stage:
            # P-matmul staging: lhsAll row0 = [s_0..s_3] chunks, row1 = ones;
            # rhsAll row0 = ones, row1 = [d_0..d_3] chunks.
            lhsAll = stage.tile([2, H * N], f32)
            rhsAll = stage.tile([2, H * N], f32)
            # diag(g_h[chunk c]) tiles: dp[c][:, h*128:(h+1)*128], bf16
            dps = [stage.tile([128, H * 128], bf16, name=f"dp{c}")
                   for c in range(NC)]
            sdg_sb = stage.tile([128, NT * 12], f32)   # sdg chunks, [i, c*12+k]

            # main-loop pools open before the prologue so the first cnt
            # loads can issue ahead of the (long) staging chain; PSUM tiles
            # allocate lazily, after the prologue psum pool has closed.
            with tc.tile_pool(name="mn", bufs=CFG["mn"]) as mn, \
                 tc.tile_pool(name="lp", bufs=CFG["lp"]) as lp, \
                 tc.tile_pool(name="mp", bufs=CFG["mp"]) as mp, \
                 tc.tile_pool(name="ob", bufs=CFG["ob"]) as obp, \
                 tc.tile_pool(name="cntp", bufs=CFG["cnt"]) as cntp:
                order = list(range(NT)) + ([0] if CFG["redo0"] else [])
                cnt_pref = {}

                def load_cnt(idx):
                    it = order[idx]
                    t = cntp.tile([128, 2 * N], bf16, tag="cnt",
                                  name=f"cnt{idx}")
                    cnap = cn_d[:, :]
                    src_ap = bass.AP(
                        tensor=cnap.tensor, offset=it * 128 * N,
                        ap=[[N, 128], [N * N, 2], [1, N]])
                    nc.sync.dma_start(
                        t.rearrange("p (b j) -> p b j", b=2), src_ap)
                    return t

                # ---- prologue ----
                with tc.tile_pool(name="pro", bufs=1) as pro, \
                     tc.tile_pool(name="pps", bufs=2, space="PSUM") as pps:
                    ones_t = pro.tile([1, N], f32)
                    nc.vector.memset(ones_t, 1.0)
                    oap1 = ones_t[0:1, :]
                    ones_b = bass.AP(tensor=oap1.tensor, offset=oap1.offset,
                                     ap=[oap1.ap[0], [0, H], oap1.ap[1]])
                    nc.sync.dma_start(lhsAll[1:2, :], ones_b)
                    nc.sync.dma_start(rhsAll[0:1, :], ones_b)
                    ident = pro.tile([128, 128], f32)
                    make_identity(nc, ident)
                    ident_bf = pro.tile([128, 128], bf16)
                    nc.vector.tensor_scalar(ident_bf, ident, 1.0, None,
                                            op0=OP.mult)
                    a_sb = pro.tile([F_IN, 12], f32)
                    nc.gpsimd.dma_start(a_sb, a_d[:, :])
                    chunks = pro.tile([128, N], f32)
                    nc.sync.dma_start(
                        chunks.rearrange("p (c f) -> p c f", c=NT),
                        src_d[:, :].rearrange("(c p) f -> p c f", p=128))
                    srcT = pro.tile([128, N], f32)
                    sdgT = pro.tile([12, N], f32)
                    cview = chunks.rearrange("p (c f) -> p c f", c=NT)
                    for c in range(NT):
                        pt = pps.tile([128, 128], f32, tag="pt")
                        nc.tensor.transpose(pt, cview[:, c, :], ident)
                        nc.scalar.copy(srcT[:, c * 128:(c + 1) * 128], pt)
                    for half in range(2):
                        ps = pps.tile([12, 512], f32, tag="sdg")
                        nc.tensor.matmul(ps, a_sb,
                                         srcT[:, half * 512:(half + 1) * 512],
                                         start=True, stop=True)
                        nc.scalar.copy(sdgT[:, half * 512:(half + 1) * 512],
                                       ps)
                    # sdg (non-transposed) chunks for per-partition g columns
                    for c in range(NT):
                        ps12 = pps.tile([128, 12], f32, tag="sdgc")
                        nc.tensor.matmul(ps12, srcT[:, c * 128:(c + 1) * 128],
                                         a_sb, start=True, stop=True)
                        nc.scalar.copy(sdg_sb[:, c * 12:(c + 1) * 12], ps12)
                    # staging rows: s chunks + d chunks
                    nc.sync.dma_start(lhsAll[0:1, :], sdgT[0:4, :])
                    nc.sync.dma_start(rhsAll[1:2, :], sdgT[4:8, :])
                    for c in range(NC):
                        for h in range(H):
                            g_col = sdg_sb[:, c * 12 + 8 + h: c * 12 + 9 + h]
                            nc.vector.tensor_scalar(
                                dps[c][:, h * 128:(h + 1) * 128],
                                ident_bf, g_col, None, op0=OP.mult)

                def lhsP(h):
                    return lhsAll[:, h * N:(h + 1) * N]

                def rhsP(h):
                    return rhsAll[:, h * N:(h + 1) * N]

                cnt_pref[0] = load_cnt(0)
                if len(order) > 1:
                    cnt_pref[1] = load_cnt(1)
                psp_cm = tc.tile_pool(name="ps", bufs=2, space="PSUM")
                psp = psp_cm.__enter__()
                tiles = [(idx, it, jb) for idx, it in enumerate(order)
                         for jb in range(N // JB)]

                # Software pipeline: stage A at tile t, B at t-1, C at t-2,
                # D at t-3.  Every instruction an engine dequeues has inputs
                # that finished >= 1 tile ago, so the in-order engine queues
                # never head-of-line block on cross-engine round trips.
                state = {}

                def stage_a(t):
                    idx, it, jb = tiles[t]
                    i0, j0 = it * 128, jb * JB
                    if jb == 0:
                        cn_t = cnt_pref.pop(idx)
                        if idx + 2 < len(order):
                            cnt_pref[idx + 2] = load_cnt(idx + 2)
                        state[("cn", idx)] = cn_t
                    cn_t = state[("cn", idx)]
                    cnt_t = cn_t[:, 0:N]
                    nct_t = cn_t[:, N:2 * N]
                    psa = psp.tile([128, H * JB], f32, tag="psa")
                    for h in range(H):
                        nc.tensor.matmul(
                            psa[:, h * JB:(h + 1) * JB],
                            lhsP(h)[:, i0:i0 + 128].bitcast(f32r),
                            rhsP(h)[:, j0:j0 + JB].bitcast(f32r),
                            start=True, stop=True)
                    for h in range(H):
                        nc.tensor.matmul(
                            psa[:, h * JB:(h + 1) * JB],
                            dps[it][:, h * 128:(h + 1) * 128],
                            cnt_t[:, j0:j0 + JB],
                            start=False, stop=True, skip_group_check=True)
                    pv = psa.rearrange("p (h j) -> p h j", h=H)
                    for c in range(JB // 128):
                        jc = jb * (JB // 128) + c
                        nc.tensor.matmul(
                            pv[:, :, c * 128:(c + 1) * 128],
                            nct_t[:, jc * 128:(jc + 1) * 128],
                            dps[jc],
                            start=False, stop=True,
                            skip_group_check=True)
                    if CFG["ablate"] >= 6:
                        return
                    l_t = lp.tile([128, H * JB], f32, tag="l")
                    if (t % 9) in CFG["dve_prelu"]:
                        nc.vector._custom_dve(
                            prelu_op, out=l_t, in0=psa, imm2=NEG_SLOPE)
                    else:
                        nc.scalar.activation(l_t, psa, AF.Prelu,
                                             alpha=NEG_SLOPE)
                    state[("l", t)] = l_t

                def stage_b(t):
                    if CFG["ablate"] >= 5:
                        return
                    l_t = state.pop(("l", t))
                    m_t = mp.tile([128, H * JB], bf16, tag="m")
                    nc.scalar.activation(m_t, l_t, AF.Exp)
                    state[("m", t)] = m_t

                def stage_e(t):
                    if CFG["ablate"] >= 4:
                        return
                    m_t = state[("m", t)]
                    s2 = mn.tile([128, 2 * JB], bf16, tag="s2")
                    s2eng = nc.gpsimd if (t % 4) in CFG["pool_s2"] else nc.vector
                    s2eng.tensor_tensor(
                        s2, m_t[:, 0:2 * JB], m_t[:, 2 * JB:4 * JB],
                        op=OP.add)
                    s_f = mn.tile([128, JB], f32, tag="s")
                    nc.gpsimd.tensor_tensor(
                        s_f, s2[:, 0:JB], s2[:, JB:2 * JB], op=OP.add)
                    state[("s", t)] = s_f

                def stage_c(t):
                    if CFG["ablate"] >= 4 or CFG["divide"]:
                        return
                    s_f = state.pop(("s", t))
                    if CFG["ablate"] >= 3:
                        return
                    # custom NR reciprocal computes in the f32 pipeline and
                    # converts to bf16 at the write, saving a convert pass
                    from concourse.dve_ops import (RECIP_APPROX_FAST_CONSTS,
                                                   RECIPROCAL_APPROX_FAST)
                    r_b = mn.tile([128, JB], bf16, tag="rb")
                    cc = RECIP_APPROX_FAST_CONSTS
                    nc.vector._custom_dve(
                        RECIPROCAL_APPROX_FAST, out=r_b[:, :], in0=s_f[:, :],
                        s0=cc["s0"], s1=cc["s1"], imm2=cc["imm2"])
                    state[("r", t)] = r_b

                def stage_d(t):
                    if CFG["ablate"] >= 3:
                        return
                    idx, it, jb = tiles[t]
                    i0, j0 = it * 128, jb * JB
                    m_t = state.pop(("m", t))
                    r_b = state.pop(("r", t))
                    rap = r_b[:, :]
                    r_b4 = bass.AP(tensor=rap.tensor, offset=rap.offset,
                                   ap=[rap.ap[0], [0, H], rap.ap[1]])
                    o_t = obp.tile([128, H * JB], bf16, tag="o")
                    o3 = o_t.rearrange("p (h j) -> p h j", h=H)
                    m3 = m_t.rearrange("p (h j) -> p h j", h=H)
                    ps = CFG["pool_mult"]
                    if ps:
                        r_b3 = bass.AP(tensor=rap.tensor, offset=rap.offset,
                                       ap=[rap.ap[0], [0, H - ps], rap.ap[1]])
                        r_b1 = bass.AP(tensor=rap.tensor, offset=rap.offset,
                                       ap=[rap.ap[0], [0, ps], rap.ap[1]])
                        nc.vector.tensor_tensor(
                            o3[:, 0:H - ps], m3[:, 0:H - ps], r_b3, op=OP.mult)
                        nc.gpsimd.tensor_tensor(
                            o3[:, H - ps:H], m3[:, H - ps:H], r_b1, op=OP.mult)
                    else:
                        nc.vector.tensor_tensor(o3, m3, r_b4, op=OP.mult)
                    if CFG["ablate"] >= 1:
                        return
                    nc.sync.dma_start(
                        out_d[:, i0:i0 + 128, j0:j0 + JB]
                        .rearrange("h p j -> p h j"),
                        o_t.rearrange("p (h j) -> p h j", h=H))

                nt_total = len(tiles)
                stage_order = CFG.get("stage_order", "abecd")
                lags = CFG.get("lags", {"a": 0, "b": 1, "e": 2, "c": 3, "d": 4})
                fns = {"a": stage_a, "b": stage_b, "e": stage_e,
                       "c": stage_c, "d": stage_d}
                maxlag = max(lags.values())
                for t in range(nt_total + maxlag):
                    for ch in stage_order:
                        if 0 <= t - lags[ch] < nt_total:
                            fns[ch](t - lags[ch])
                psp_cm.__exit__(None, None, None)
    nc.finalize()
    return nc


def _prepare_in_maps(src, edge_index, W_lin, a_src, a_dst, W_edge, a_edge):
    import ml_dtypes

    src = np.ascontiguousarray(np.asarray(src, dtype=np.float32))
    ei = np.asarray(edge_index).astype(np.int64)
    W_lin = np.asarray(W_lin, dtype=np.float32)
    a_src = np.asarray(a_src, dtype=np.float32)
    a_dst = np.asarray(a_dst, dtype=np.float32)
    W_edge = np.asarray(W_edge, dtype=np.float32)
    a_edge = np.asarray(a_edge, dtype=np.float32)

    # fold weights: A = [W_lin@a_src | W_lin@a_dst | W_edge@a_edge]  [128,12]
    A = np.concatenate(
        [W_lin @ a_src, W_lin @ a_dst, W_edge @ a_edge], axis=1
    ).astype(np.float32)
    # edge multiplicity matrix (shared across batches)
    cnt = np.zeros((N, N), np.float32)
    np.add.at(cnt, (ei[0], ei[1]), 1.0)
    # -cnt^T packed per-i-tile: ncntT[it*128+p, q*128+f] = -cnt[it*128+f, q*128+p]
    T = np.ascontiguousarray((-cnt).T)          # T[j, i] = -cnt[i, j]
    ncntT = T.reshape(NT, 128, NT, 128).transpose(2, 1, 0, 3).reshape(N, N)
    cn = np.ascontiguousarray(
        np.concatenate([cnt, ncntT], axis=0)).astype(ml_dtypes.bfloat16)
    return [
        {"src": np.ascontiguousarray(src[b]), "cn": cn, "A": A}
        for b in range(B)
    ]


def kernel(src, edge_index, W_lin, a_src, a_dst, W_edge, a_edge):
    from concourse.bass_utils import run_bass_kernel_spmd

    in_maps = _prepare_in_maps(src, edge_index, W_lin, a_src, a_dst,
                               W_edge, a_edge)
    nc = _build_nc()
    res = run_bass_kernel_spmd(nc, in_maps, core_ids=list(range(B)))
    out = np.stack(
        [np.asarray(res.results[b]["out"]).astype(np.float32)
         .transpose(1, 2, 0) for b in range(B)], axis=0)
    return np.ascontiguousarray(out)


if __name__ == "__main__":
    rng = np.random.default_rng(0)
    inputs = {
        "src": rng.standard_normal((B, N, F_IN), dtype=np.float32),
        "edge_index": rng.integers(0, N, (2, 32768)).astype(np.int32),
        "W_lin": rng.standard_normal((F_IN, 128), dtype=np.float32) / np.sqrt(F_IN),
        "a_src": rng.standard_normal((128, H), dtype=np.float32) / np.sqrt(128),
        "a_dst": rng.standard_normal((128, H), dtype=np.float32) / np.sqrt(128),
        "W_edge": rng.standard_normal((F_IN, 64), dtype=np.float32) / np.sqrt(F_IN),
        "a_edge": rng.standard_normal((64, H), dtype=np.float32) / np.sqrt(64),
    }
    out = kernel(**inputs)
    print("out", out.shape, out.dtype, out.sum())
